# revision 15
# baseline (speedup 1.0000x reference)
"""BRITS bidirectional-LSTM imputation kernel for Trainium2 (Bass/Tile).

Sharding: 8 cores = 2 directions x 4 batch shards of 64.
Each core runs its own T-step sequential scan (batch=64, feature-major layout).

On-chip layout: activations are feature-major [feat_partition, batch_free].
All matmuls are weight-stationary bf16 (FWL weight loads), accumulate f32 PSUM.
H=256-sized vectors are stored as [128 part, 2 chunks * 64 batch]; the gate
output (4H=1024) lives in one PSUM bank [128, 8*64] with gate order permuted
to [i, f, o, g] so sigmoid covers a contiguous [128, 384] block.

Phase P precomputes (time-batched, before the scan) the h-independent terms:
  rr   = min(exp(-(lagW @ lt + lag_b)), 1)            -> rr_sb   (bf16, SBUF)
  rb   = rbetaW @ rr + rbeta_b
  beta = testW @ sigmoid(betaW @ [m; rb] + beta_b) + test_b -> beta_sb
This keeps Exp out of the scan loop so the loop runs on a single activation
table set (sigmoid/tanh) with no LoadActFuncSet churn.

All affine biases inside the loop are injected into PSUM via tiny bias-row
matmuls (K=1 ones trick / K=8 selector for the gates), so the loop's ACT ops
are only Sigmoid/Tanh.
"""

import os
from contextlib import ExitStack

import numpy as np
import ml_dtypes

B, F, H = 256, 128, 256
NCORES = 8
BL = 64  # batch per core (2 dirs x 4 shards)
CHUNK = 32  # steps per chunk (phase P batching + input streaming)
NFILL = int(os.environ.get("NFILL", "28"))  # PE warm-keeper ldweights per step

_BF = ml_dtypes.bfloat16

_BUILD_CACHE = {}


def _build(T):
    """Build the Bass program for a T-step scan. Same program for all cores."""
    import concourse.tile as tile
    import concourse.mybir as mybir
    from concourse import bacc

    f32 = mybir.dt.float32
    bf16 = mybir.dt.bfloat16
    u8 = mybir.dt.uint8
    AF = mybir.ActivationFunctionType
    ALU = mybir.AluOpType

    nc = bacc.Bacc("TRN2", target_bir_lowering=False, debug=False)

    NCH = min(CHUNK, T)
    NJ = (T + NCH - 1) // NCH
    W = NCH * BL  # free width of a step-chunk

    # ---- DRAM I/O (per core) ----
    xt = nc.dram_tensor("xt", [F, T, BL], bf16, kind="ExternalInput")
    mt = nc.dram_tensor("mt", [F, T, BL], bf16, kind="ExternalInput")
    lt = nc.dram_tensor("lt", [F, T, BL], bf16, kind="ExternalInput")

    linWT = nc.dram_tensor("linWT", [H, F], bf16, kind="ExternalInput")
    zodT = nc.dram_tensor("zodT", [F, F], bf16, kind="ExternalInput")
    WihT = nc.dram_tensor("WihT", [2 * F, 4 * H], bf16, kind="ExternalInput")
    WhhT = nc.dram_tensor("WhhT", [H, 4 * H], bf16, kind="ExternalInput")
    bias8 = nc.dram_tensor("bias8", [8, F], bf16, kind="ExternalInput")
    sel8 = nc.dram_tensor("sel8", [8, 8 * BL], bf16, kind="ExternalInput")
    lagWT = nc.dram_tensor("lagWT", [F, H], bf16, kind="ExternalInput")
    nlagb = nc.dram_tensor("nlagb", [F, 2], f32, kind="ExternalInput")
    rbetaWT = nc.dram_tensor("rbetaWT", [H, F], bf16, kind="ExternalInput")
    rbeta_b = nc.dram_tensor("rbeta_b", [F, 1], f32, kind="ExternalInput")
    betaWT = nc.dram_tensor("betaWT", [2 * F, 3 * F], bf16, kind="ExternalInput")
    beta_b = nc.dram_tensor("beta_b", [F, 3], f32, kind="ExternalInput")
    testWT = nc.dram_tensor("testWT", [3 * F, F], bf16, kind="ExternalInput")
    test_b = nc.dram_tensor("test_b", [F, 1], f32, kind="ExternalInput")
    linb_row = nc.dram_tensor("linb_row", [1, F], bf16, kind="ExternalInput")
    zb_row = nc.dram_tensor("zb_row", [1, F], bf16, kind="ExternalInput")

    o_out = nc.dram_tensor("o_out", [T, F, BL], f32, kind="ExternalOutput")
    z_out = nc.dram_tensor("z_out", [T, F, BL], f32, kind="ExternalOutput")
    c_out = nc.dram_tensor("c_out", [T, F, BL], f32, kind="ExternalOutput")

    with tile.TileContext(nc) as tc, ExitStack() as ctx:
        consts = ctx.enter_context(tc.tile_pool(name="consts", bufs=1))

        # ---- load weights to SBUF ----
        linW_sb = consts.tile([128, 2, F], bf16)
        for k in range(2):
            nc.sync.dma_start(out=linW_sb[:, k, :], in_=linWT[k * 128:(k + 1) * 128, :])
        zod_sb = consts.tile([128, F], bf16)
        nc.sync.dma_start(out=zod_sb[:], in_=zodT[:, :])
        Wih_sb = consts.tile([128, 2, 4 * H], bf16)
        Whh_sb = consts.tile([128, 2, 4 * H], bf16)
        for k in range(2):
            nc.sync.dma_start(out=Wih_sb[:, k, :], in_=WihT[k * 128:(k + 1) * 128, :])
            nc.sync.dma_start(out=Whh_sb[:, k, :], in_=WhhT[k * 128:(k + 1) * 128, :])
        bias8_sb = consts.tile([8, F], bf16)
        nc.sync.dma_start(out=bias8_sb[:], in_=bias8[:, :])
        sel8_sb = consts.tile([8, 8 * BL], bf16)
        nc.sync.dma_start(out=sel8_sb[:], in_=sel8[:, :])
        lagW_sb = consts.tile([128, H], bf16)
        nc.sync.dma_start(out=lagW_sb[:], in_=lagWT[:, :])
        rbetaW_sb = consts.tile([128, 2, F], bf16)
        for k in range(2):
            nc.sync.dma_start(out=rbetaW_sb[:, k, :], in_=rbetaWT[k * 128:(k + 1) * 128, :])
        betaW_sb = consts.tile([128, 2, 3 * F], bf16)
        for k in range(2):
            nc.sync.dma_start(out=betaW_sb[:, k, :], in_=betaWT[k * 128:(k + 1) * 128, :])
        testW_sb = consts.tile([128, 3, F], bf16)
        for k in range(3):
            nc.sync.dma_start(out=testW_sb[:, k, :], in_=testWT[k * 128:(k + 1) * 128, :])

        nlagb_sb = consts.tile([128, 2], f32)
        nc.sync.dma_start(out=nlagb_sb[:], in_=nlagb[:, :])
        rbetab_sb = consts.tile([128, 1], f32)
        nc.sync.dma_start(out=rbetab_sb[:], in_=rbeta_b[:, :])
        betab_sb = consts.tile([128, 3], f32)
        nc.sync.dma_start(out=betab_sb[:], in_=beta_b[:, :])
        testb_sb = consts.tile([128, 1], f32)
        nc.sync.dma_start(out=testb_sb[:], in_=test_b[:, :])
        linbr_sb = consts.tile([1, F], bf16)
        nc.sync.dma_start(out=linbr_sb[:], in_=linb_row[:, :])
        zbr_sb = consts.tile([1, F], bf16)
        nc.sync.dma_start(out=zbr_sb[:], in_=zb_row[:, :])
        ones1 = consts.tile([1, BL], bf16)
        nc.vector.memset(ones1[:], 1.0)
        fill_w = consts.tile([128, 128], bf16)
        nc.vector.memset(fill_w[:], 0.0)

        # ---- SBUF-resident phase-P products ----
        big = ctx.enter_context(tc.tile_pool(name="big", bufs=1))
        rr_sb = [big.tile([128, NCH * 2 * BL], bf16, tag=f"rr{j}", name=f"rr_sb{j}") for j in range(NJ)]
        beta_sb = [big.tile([128, NCH * BL], bf16, tag=f"bt{j}", name=f"beta_sb{j}") for j in range(NJ)]

        # =========================== PHASE P ===========================
        with ExitStack() as pctx:
            pstr = pctx.enter_context(tc.tile_pool(name="pstr", bufs=2))
            pwork = pctx.enter_context(tc.tile_pool(name="pwork", bufs=2))
            pps = pctx.enter_context(tc.tile_pool(name="pps", bufs=2, space="PSUM"))

            for j in range(NJ):
                t0 = j * NCH
                lt_ch = pstr.tile([128, NCH, BL], bf16, tag="lt_ch")
                mp_ch = pstr.tile([128, NCH, BL], bf16, tag="mp_ch")
                nc.sync.dma_start(out=lt_ch[:], in_=lt[:, t0:t0 + NCH, :])
                nc.sync.dma_start(out=mp_ch[:], in_=mt[:, t0:t0 + NCH, :])

                NH = W // 2  # half-chunk free width (psum tile size)
                SPH = NCH // 2  # steps per half
                NQ = max(1, NH // 512)  # matmul N sub-tiles per psum tile
                QW = NH // NQ  # sub-tile width (<= 512)
                SQ = QW // BL  # steps per sub-tile

                # rr = min(exp(-(lagW@lt + lag_b)), 1), M-tiles k=0,1
                for k in range(2):
                    for h in range(2):
                        ps = pps.tile([128, NH], f32, tag="pp")
                        for q in range(NQ):
                            nc.tensor.matmul(
                                out=ps[:, q * QW:(q + 1) * QW],
                                lhsT=lagW_sb[:, k * 128:(k + 1) * 128],
                                rhs=lt_ch[:, h * SPH + q * SQ:h * SPH + (q + 1) * SQ, :],
                            )
                        e_ch = pwork.tile([128, NH], bf16, tag="e_ch")
                        nc.scalar.activation(e_ch[:], ps[:], AF.Exp,
                                             bias=nlagb_sb[:, k:k + 1], scale=-1.0)
                        dst = rr_sb[j][:]
                        dst = dst.rearrange("p (t c b) -> p t c b", c=2, b=BL)
                        nc.vector.tensor_scalar_min(
                            dst[:, h * SPH:(h + 1) * SPH, k, :],
                            e_ch[:].rearrange("p (t b) -> p t b", b=BL), 1.0)

                # rb = rbetaW @ rr + rbeta_b  (plain [t*64+b] layout)
                rb_ch = pwork.tile([128, NCH, BL], bf16, tag="rb_ch")
                rrv = rr_sb[j][:].rearrange("p (t c b) -> p t c b", c=2, b=BL)
                for h in range(2):
                    ps = pps.tile([128, NH], f32, tag="pp")
                    for q in range(NQ):
                        for k in range(2):
                            nc.tensor.matmul(
                                out=ps[:, q * QW:(q + 1) * QW],
                                lhsT=rbetaW_sb[:, k, :],
                                rhs=rrv[:, h * SPH + q * SQ:h * SPH + (q + 1) * SQ, k, :],
                                start=(k == 0), stop=(k == 1),
                            )
                    nc.vector.tensor_scalar_add(
                        rb_ch[:, h * SPH:(h + 1) * SPH, :],
                        ps[:].rearrange("p (t b) -> p t b", b=BL),
                        rbetab_sb[:, 0:1])

                # s3 = sigmoid(betaW @ [m; rb] + beta_b), M-tiles m3=0..2
                s3_ch = pwork.tile([128, 3, NCH, BL], bf16, tag="s3_ch")
                for m3 in range(3):
                    for h in range(2):
                        ps = pps.tile([128, NH], f32, tag="pp")
                        for q in range(NQ):
                            for k, rsrc in ((0, mp_ch), (1, rb_ch)):
                                nc.tensor.matmul(
                                    out=ps[:, q * QW:(q + 1) * QW],
                                    lhsT=betaW_sb[:, k, m3 * 128:(m3 + 1) * 128],
                                    rhs=rsrc[:, h * SPH + q * SQ:h * SPH + (q + 1) * SQ, :],
                                    start=(k == 0), stop=(k == 1),
                                )
                        nc.scalar.activation(
                            s3_ch[:, m3, h * SPH:(h + 1) * SPH, :],
                            ps[:].rearrange("p (t b) -> p t b", b=BL),
                            AF.Sigmoid, bias=betab_sb[:, m3:m3 + 1], scale=1.0)

                # beta = testW @ s3 + test_b
                btv = beta_sb[j][:].rearrange("p (t b) -> p t b", b=BL)
                for h in range(2):
                    ps = pps.tile([128, NH], f32, tag="pp")
                    for q in range(NQ):
                        for k in range(3):
                            nc.tensor.matmul(
                                out=ps[:, q * QW:(q + 1) * QW],
                                lhsT=testW_sb[:, k, :],
                                rhs=s3_ch[:, k, h * SPH + q * SQ:h * SPH + (q + 1) * SQ, :],
                                start=(k == 0), stop=(k == 2),
                            )
                    nc.vector.tensor_scalar_add(
                        btv[:, h * SPH:(h + 1) * SPH, :],
                        ps[:].rearrange("p (t b) -> p t b", b=BL),
                        testb_sb[:, 0:1])

        # =========================== PHASE L ===========================
        state = ctx.enter_context(tc.tile_pool(name="state", bufs=2))
        h_bf = state.tile([128, 2 * BL], bf16, tag="h")
        c_f = state.tile([128, 2 * BL], f32, tag="c")
        hr_bf = state.tile([128, 2 * BL], bf16, tag="hr")
        nc.vector.memset(h_bf[:], 0.0)
        nc.vector.memset(c_f[:], 0.0)
        nc.vector.memset(hr_bf[:], 0.0)

        streams = ctx.enter_context(tc.tile_pool(name="streams", bufs=2))
        work = ctx.enter_context(tc.tile_pool(name="work", bufs=3))
        psg_pool = ctx.enter_context(tc.tile_pool(name="psg", bufs=2, space="PSUM"))
        psl_pool = ctx.enter_context(tc.tile_pool(name="psl", bufs=3, space="PSUM"))
        psz_pool = ctx.enter_context(tc.tile_pool(name="psz", bufs=3, space="PSUM"))

        def make_chunk(j):
            """DMA + mask algebra for chunk j (issued ahead of its use)."""
            t0 = j * NCH
            x_ch = streams.tile([128, NCH, BL], bf16, tag="x_ch")
            m_ch = streams.tile([128, NCH, BL], bf16, tag="m_ch")
            nc.sync.dma_start(out=x_ch[:], in_=xt[:, t0:t0 + NCH, :])
            nc.sync.dma_start(out=m_ch[:], in_=mt[:, t0:t0 + NCH, :])
            im_ch = streams.tile([128, NCH, BL], bf16, tag="im_ch")
            nc.vector.tensor_scalar(im_ch[:], m_ch[:], -1.0, 1.0,
                                    mybir.AluOpType.mult, mybir.AluOpType.add)
            xm_ch = streams.tile([128, NCH, BL], bf16, tag="xm_ch")
            nc.vector.tensor_mul(xm_ch[:], m_ch[:], x_ch[:])
            bview = beta_sb[j][:].rearrange("p (t b) -> p t b", b=BL)
            ib_ch = streams.tile([128, NCH, BL], bf16, tag="ib_ch")
            nc.vector.tensor_mul(ib_ch[:], im_ch[:], bview)
            a_ch = streams.tile([128, NCH, BL], bf16, tag="a_ch")
            nc.vector.tensor_sub(a_ch[:], im_ch[:], ib_ch[:])
            return (m_ch, im_ch, xm_ch, a_ch, ib_ch)

        cur_ch = make_chunk(0)
        nxt_ch = None

        for t in range(T):
            j, tl = t // NCH, t % NCH
            if tl == 0 and t > 0:
                cur_ch = nxt_ch
            if tl == NCH - 6 and j + 1 < NJ:
                nxt_ch = make_chunk(j + 1)
            m_ch, im_ch, xm_ch, a_ch, ib_ch = cur_ch
            ms = m_ch[:, tl, :]
            im_s = im_ch[:, tl, :]
            xm_s = xm_ch[:, tl, :]
            a_s = a_ch[:, tl, :]
            ib_s = ib_ch[:, tl, :]
            rr_t = rr_sb[j][:, tl * 2 * BL:(tl + 1) * 2 * BL]
            beta_t = beta_sb[j][:, tl * BL:(tl + 1) * BL]

            # ---- out = linW @ h + lin_b  (bias via K=1 ones-row matmul) ----
            ps_lin = psl_pool.tile([128, BL], f32, tag="ps_lin")
            nc.tensor.matmul(out=ps_lin[:], lhsT=linbr_sb[:], rhs=ones1[:],
                             start=True, stop=False, skip_group_check=True)
            for k in range(2):
                nc.tensor.matmul(
                    out=ps_lin[:], lhsT=linW_sb[:, k, :],
                    rhs=h_bf[:, k * BL:(k + 1) * BL],
                    start=False, stop=(k == 1), skip_group_check=True,
                )

            # ---- x_c = m*x + (1-m)*out ; z = zod @ x_c + z_b ----
            # z is accumulated as zod@xm (early) + zod@((1-m)*out) (late)
            ps_z = psz_pool.tile([128, BL], f32, tag="ps_z")
            nc.tensor.matmul(out=ps_z[:], lhsT=zbr_sb[:], rhs=ones1[:],
                             start=True, stop=False, skip_group_check=True)
            nc.tensor.matmul(out=ps_z[:], lhsT=zod_sb[:], rhs=xm_s,
                             start=False, stop=False, skip_group_check=True)
            xc0 = work.tile([128, BL], bf16, tag="xc0")
            nc.vector.tensor_mul(xc0[:], im_s, ps_lin[:])
            nc.tensor.matmul(out=ps_z[:], lhsT=zod_sb[:], rhs=xc0[:],
                             start=False, stop=True, skip_group_check=True)

            # ---- c_c = xm + a*out + ib*z  as two gate K-tiles w2, w3 ----
            w1 = work.tile([128, BL], f32, tag="w1")
            nc.vector.tensor_mul(w1[:], a_s, ps_lin[:])
            w2 = work.tile([128, BL], bf16, tag="w2")
            nc.vector.tensor_add(w2[:], w1[:], xm_s)
            w3 = work.tile([128, BL], bf16, tag="w3")
            nc.vector.tensor_mul(w3[:], ib_s, ps_z[:])

            # ---- off-chain: out/z to SBUF, c_val = out + beta*(z-out) ----
            out_f = work.tile([128, BL], f32, tag="out_f")
            nc.vector.tensor_copy(out_f[:], ps_lin[:])
            z_f = work.tile([128, BL], f32, tag="z_f")
            nc.scalar.activation(z_f[:], ps_z[:], AF.Copy)
            d_f = work.tile([128, BL], f32, tag="d_f")
            nc.vector.tensor_sub(d_f[:], ps_z[:], out_f[:])
            e_f = work.tile([128, BL], f32, tag="e_f")
            nc.gpsimd.tensor_mul(e_f[:], beta_t, d_f[:])
            cval_f = work.tile([128, BL], f32, tag="cval_f")
            nc.gpsimd.tensor_add(cval_f[:], e_f[:], out_f[:])

            # ---- gates = bias + Wih@[cc; m] + Whh@h_rr  (order i,f,o,g) ----
            ps_g = psg_pool.tile([128, 8 * BL], f32, tag="ps_g")
            nc.tensor.matmul(out=ps_g[:], lhsT=bias8_sb[:], rhs=sel8_sb[:],
                             start=True, stop=False, skip_group_check=True)
            grhs = (ms, hr_bf[:, 0:BL], hr_bf[:, BL:2 * BL], w2[:], w3[:])
            glhs = (Wih_sb[:, 1, :], Whh_sb[:, 0, :], Whh_sb[:, 1, :],
                    Wih_sb[:, 0, :], Wih_sb[:, 0, :])
            for k in range(5):
                for mc in range(8):
                    nc.tensor.matmul(
                        out=ps_g[:, mc * BL:(mc + 1) * BL],
                        lhsT=glhs[k][:, mc * 128:(mc + 1) * 128],
                        rhs=grhs[k], start=False, stop=(k == 4),
                        skip_group_check=True,
                    )

            # ---- PE warm-keepers: dep-free weight loads that run during the
            # activation/DVE tail so the HAM clock gate stays at full rate ----
            for _ in range(NFILL):
                nc.tensor.ldweights(weights=fill_w[:])

            # ---- gate activations: i,f first (critical), then g, o ----
            sif = work.tile([128, 4 * BL], f32, tag="sif")
            nc.scalar.activation(sif[:], ps_g[:, 0:4 * BL], AF.Sigmoid)
            gt = work.tile([128, 2 * BL], f32, tag="gt")
            nc.scalar.activation(gt[:], ps_g[:, 6 * BL:8 * BL], AF.Tanh)
            so = work.tile([128, 2 * BL], f32, tag="so")
            nc.scalar.activation(so[:], ps_g[:, 4 * BL:6 * BL], AF.Sigmoid)

            # ---- c' = sig_f*c + sig_i*tanh(g);  h' = sig_o*tanh(c') ----
            t1 = work.tile([128, 2 * BL], f32, tag="t1")
            nc.vector.tensor_mul(t1[:], sif[:, 2 * BL:4 * BL], c_f[:])
            t2 = work.tile([128, 2 * BL], f32, tag="t2")
            nc.vector.tensor_mul(t2[:], sif[:, 0:2 * BL], gt[:])
            c_new = state.tile([128, 2 * BL], f32, tag="c")
            nc.vector.tensor_add(c_new[:], t1[:], t2[:])
            tc2 = work.tile([128, 2 * BL], f32, tag="tc2")
            nc.scalar.activation(tc2[:], c_new[:], AF.Tanh)
            h_new = state.tile([128, 2 * BL], bf16, tag="h")
            nc.vector.tensor_mul(h_new[:], so[:], tc2[:])
            if t + 1 < T:
                jn, tn = (t + 1) // NCH, (t + 1) % NCH
                rr_n = rr_sb[jn][:, tn * 2 * BL:(tn + 1) * 2 * BL]
                or2 = work.tile([128, 2 * BL], f32, tag="or2")
                nc.gpsimd.tensor_mul(or2[:], so[:], rr_n)
                hr_new = state.tile([128, 2 * BL], bf16, tag="hr")
                nc.gpsimd.tensor_mul(hr_new[:], or2[:], tc2[:])
                hr_bf = hr_new

            # ---- store outputs ----
            nc.sync.dma_start(out=o_out[t], in_=out_f[:])
            nc.sync.dma_start(out=z_out[t], in_=z_f[:])
            nc.sync.dma_start(out=c_out[t], in_=cval_f[:])

            h_bf = h_new
            c_f = c_new

    nc.compile()
    return nc


def _prep_weights(inputs, d):
    """Host-side weight layout prep for direction d (0=fw, 1=bw). bf16."""
    p = "fw" if d == 0 else "bw"
    Wih = np.asarray(inputs[f"{p}_Wih"], np.float32)
    Whh = np.asarray(inputs[f"{p}_Whh"], np.float32)
    bih = np.asarray(inputs[f"{p}_bih"], np.float32)
    bhh = np.asarray(inputs[f"{p}_bhh"], np.float32)
    lin_W = np.asarray(inputs[f"{p}lin_W"], np.float32)
    lin_b = np.asarray(inputs[f"{p}lin_b"], np.float32)
    z_W = np.asarray(inputs[f"{p}z_W"], np.float32)
    z_b = np.asarray(inputs[f"{p}z_b"], np.float32)
    beta_W = np.asarray(inputs[f"{p}beta_W"], np.float32)
    beta_b = np.asarray(inputs[f"{p}beta_b"], np.float32)
    lag_W = np.asarray(inputs["lag_W" if d == 0 else "lagb_W"], np.float32)
    lag_b = np.asarray(inputs["lag_b" if d == 0 else "lagb_b"], np.float32)
    rbeta_W = np.asarray(inputs["rbeta_W" if d == 0 else "rbetab_W"], np.float32)
    rbeta_b = np.asarray(inputs["rbeta_b" if d == 0 else "rbetab_b"], np.float32)
    test_W = np.asarray(inputs["test_W"], np.float32)
    test_b = np.asarray(inputs["test_b"], np.float32)

    perm = np.concatenate([np.arange(0, 512), np.arange(768, 1024),
                           np.arange(512, 768)])
    sel8 = np.zeros((8, 8 * BL), np.float32)
    for j in range(8):
        sel8[j, j * BL:(j + 1) * BL] = 1.0

    def c(a):
        return np.ascontiguousarray(a)

    w = {
        "linWT": c(lin_W.T).astype(_BF),
        "zodT": c((z_W * (1.0 - np.eye(F, dtype=np.float32))).T).astype(_BF),
        "WihT": c(Wih[perm].T).astype(_BF),
        "WhhT": c(Whh[perm].T).astype(_BF),
        "bias8": c((bih + bhh)[perm].reshape(8, F)).astype(_BF),
        "sel8": sel8.astype(_BF),
        "lagWT": c(lag_W.T).astype(_BF),
        "nlagb": c((-lag_b).reshape(2, F).T).astype(np.float32),
        "rbetaWT": c(rbeta_W.T).astype(_BF),
        "rbeta_b": c(rbeta_b.reshape(F, 1)).astype(np.float32),
        "betaWT": c(beta_W.T).astype(_BF),
        "beta_b": c(beta_b.reshape(3, F).T).astype(np.float32),
        "testWT": c(test_W.T).astype(_BF),
        "test_b": c(test_b.reshape(F, 1)).astype(np.float32),
        "linb_row": c(lin_b.reshape(1, F)).astype(_BF),
        "zb_row": c(z_b.reshape(1, F)).astype(_BF),
    }
    return w


def _make_in_maps(inputs, T):
    x = np.asarray(inputs["x"], np.float32)
    m = np.asarray(inputs["masking"], np.float32)
    tl = np.asarray(inputs["time_lag"], np.float32)
    Bfull = x.shape[0]
    nsh = Bfull // BL

    xt = np.ascontiguousarray(x.transpose(2, 1, 0)).astype(_BF)
    mt = np.ascontiguousarray(m.transpose(2, 1, 0)).astype(_BF)
    ltt = np.ascontiguousarray(tl.transpose(2, 1, 0)).astype(_BF)
    xtr = np.ascontiguousarray(xt[:, ::-1, :])
    mtr = np.ascontiguousarray(mt[:, ::-1, :])
    lttr = np.ascontiguousarray(ltt[:, ::-1, :])

    wts = [_prep_weights(inputs, 0), _prep_weights(inputs, 1)]

    in_maps = []
    for core in range(NCORES):
        d, s = core // nsh, core % nsh
        sl = slice(s * BL, (s + 1) * BL)
        if d == 0:
            im = {"xt": np.ascontiguousarray(xt[:, :, sl]),
                  "mt": np.ascontiguousarray(mt[:, :, sl]),
                  "lt": np.ascontiguousarray(ltt[:, :, sl])}
        else:
            im = {"xt": np.ascontiguousarray(xtr[:, :, sl]),
                  "mt": np.ascontiguousarray(mtr[:, :, sl]),
                  "lt": np.ascontiguousarray(lttr[:, :, sl])}
        im.update(wts[d])
        in_maps.append(im)
    return in_maps, nsh


def _gather(res, T, Bfull, nsh):
    outs = []
    for d in range(2):
        o = np.empty((Bfull, T, F), np.float32)
        z = np.empty((Bfull, T, F), np.float32)
        cv = np.empty((Bfull, T, F), np.float32)
        for s in range(nsh):
            r = res[d * nsh + s]
            sl = slice(s * BL, (s + 1) * BL)
            ro, rz, rc = r["o_out"], r["z_out"], r["c_out"]
            if d == 1:
                ro, rz, rc = ro[::-1], rz[::-1], rc[::-1]
            o[sl] = ro.transpose(2, 0, 1)
            z[sl] = rz.transpose(2, 0, 1)
            cv[sl] = rc.transpose(2, 0, 1)
        outs += [o, z, cv]
    return tuple(outs)


def _run(inputs, T, trace=False):
    from concourse.bass_utils import run_bass_kernel_spmd

    if T not in _BUILD_CACHE:
        _BUILD_CACHE[T] = _build(T)
    nc = _BUILD_CACHE[T]
    in_maps, nsh = _make_in_maps(inputs, T)
    br = run_bass_kernel_spmd(nc, in_maps, core_ids=list(range(NCORES)),
                              trace=trace)
    Bfull = np.asarray(inputs["x"]).shape[0]
    return _gather(br.results, T, Bfull, nsh), br


def kernel(**inputs):
    T = np.asarray(inputs["x"]).shape[1]
    outs, _ = _run(inputs, T, trace=False)
    return outs



# revision 19
# speedup vs baseline: 1.2478x; 1.2478x over previous
"""BRITS bidirectional-LSTM imputation kernel for Trainium2 (Bass/Tile).

Sharding: 8 cores = 2 directions x 4 batch shards of 64.
Each core runs its own T-step sequential scan (batch=64, feature-major layout).

On-chip layout: activations are feature-major [feat_partition, batch_free].
All matmuls are weight-stationary bf16 (FWL weight loads), accumulate f32 PSUM.
H=256-sized vectors are stored as [128 part, 2 chunks * 64 batch]; the gate
output (4H=1024) lives in one PSUM bank [128, 8*64] with gate order permuted
to [i, f, o, g] so sigmoid covers a contiguous [128, 384] block.

Phase P precomputes (time-batched, before the scan) the h-independent terms:
  rr   = min(exp(-(lagW @ lt + lag_b)), 1)            -> rr_sb   (bf16, SBUF)
  rb   = rbetaW @ rr + rbeta_b
  beta = testW @ sigmoid(betaW @ [m; rb] + beta_b) + test_b -> beta_sb
This keeps Exp out of the scan loop so the loop runs on a single activation
table set (sigmoid/tanh) with no LoadActFuncSet churn.

All affine biases inside the loop are injected into PSUM via tiny bias-row
matmuls (K=1 ones trick / K=8 selector for the gates), so the loop's ACT ops
are only Sigmoid/Tanh.
"""

import os
from contextlib import ExitStack

import numpy as np
import ml_dtypes

B, F, H = 256, 128, 256
NCORES = 8
BL = 64  # batch per core (2 dirs x 4 shards)
CHUNK = 32  # steps per chunk (phase P batching + input streaming)
NFILL = int(os.environ.get("NFILL", "0"))  # PE warm-keeper ldweights per step

_BF = ml_dtypes.bfloat16

_BUILD_CACHE = {}


def _build(T):
    """Build the Bass program for a T-step scan. Same program for all cores."""
    import concourse.tile as tile
    import concourse.mybir as mybir
    from concourse import bacc

    f32 = mybir.dt.float32
    bf16 = mybir.dt.bfloat16
    u8 = mybir.dt.uint8
    AF = mybir.ActivationFunctionType
    ALU = mybir.AluOpType

    nc = bacc.Bacc("TRN2", target_bir_lowering=False, debug=False)

    NCH = min(CHUNK, T)
    NJ = (T + NCH - 1) // NCH
    W = NCH * BL  # free width of a step-chunk

    # ---- DRAM I/O (per core) ----
    xt = nc.dram_tensor("xt", [F, T, BL], bf16, kind="ExternalInput")
    mt = nc.dram_tensor("mt", [F, T, BL], bf16, kind="ExternalInput")
    lt = nc.dram_tensor("lt", [F, T, BL], bf16, kind="ExternalInput")

    linWT = nc.dram_tensor("linWT", [H, F], bf16, kind="ExternalInput")
    zodT = nc.dram_tensor("zodT", [F, F], bf16, kind="ExternalInput")
    WihT = nc.dram_tensor("WihT", [2 * F, 4 * H], bf16, kind="ExternalInput")
    WhhT = nc.dram_tensor("WhhT", [H, 4 * H], bf16, kind="ExternalInput")
    bias8 = nc.dram_tensor("bias8", [8, F], bf16, kind="ExternalInput")
    sel8 = nc.dram_tensor("sel8", [8, 8 * BL], bf16, kind="ExternalInput")
    lagWT = nc.dram_tensor("lagWT", [F, H], bf16, kind="ExternalInput")
    nlagb = nc.dram_tensor("nlagb", [F, 2], f32, kind="ExternalInput")
    rbetaWT = nc.dram_tensor("rbetaWT", [H, F], bf16, kind="ExternalInput")
    rbeta_b = nc.dram_tensor("rbeta_b", [F, 1], f32, kind="ExternalInput")
    betaWT = nc.dram_tensor("betaWT", [2 * F, 3 * F], bf16, kind="ExternalInput")
    beta_b = nc.dram_tensor("beta_b", [F, 3], f32, kind="ExternalInput")
    testWT = nc.dram_tensor("testWT", [3 * F, F], bf16, kind="ExternalInput")
    test_b = nc.dram_tensor("test_b", [F, 1], f32, kind="ExternalInput")
    linb_row = nc.dram_tensor("linb_row", [1, F], bf16, kind="ExternalInput")
    zb_row = nc.dram_tensor("zb_row", [1, F], bf16, kind="ExternalInput")

    o_out = nc.dram_tensor("o_out", [T, F, BL], f32, kind="ExternalOutput")
    z_out = nc.dram_tensor("z_out", [T, F, BL], f32, kind="ExternalOutput")
    c_out = nc.dram_tensor("c_out", [T, F, BL], f32, kind="ExternalOutput")

    with tile.TileContext(nc) as tc, ExitStack() as ctx:
        consts = ctx.enter_context(tc.tile_pool(name="consts", bufs=1))

        # ---- load weights to SBUF ----
        linW_sb = consts.tile([128, 2, F], bf16)
        for k in range(2):
            nc.sync.dma_start(out=linW_sb[:, k, :], in_=linWT[k * 128:(k + 1) * 128, :])
        zod_sb = consts.tile([128, F], bf16)
        nc.sync.dma_start(out=zod_sb[:], in_=zodT[:, :])
        Wih_sb = consts.tile([128, 2, 4 * H], bf16)
        Whh_sb = consts.tile([128, 2, 4 * H], bf16)
        for k in range(2):
            nc.sync.dma_start(out=Wih_sb[:, k, :], in_=WihT[k * 128:(k + 1) * 128, :])
            nc.sync.dma_start(out=Whh_sb[:, k, :], in_=WhhT[k * 128:(k + 1) * 128, :])
        bias8_sb = consts.tile([8, F], bf16)
        nc.sync.dma_start(out=bias8_sb[:], in_=bias8[:, :])
        sel8_sb = consts.tile([8, 8 * BL], bf16)
        nc.sync.dma_start(out=sel8_sb[:], in_=sel8[:, :])
        lagW_sb = consts.tile([128, H], bf16)
        nc.sync.dma_start(out=lagW_sb[:], in_=lagWT[:, :])
        rbetaW_sb = consts.tile([128, 2, F], bf16)
        for k in range(2):
            nc.sync.dma_start(out=rbetaW_sb[:, k, :], in_=rbetaWT[k * 128:(k + 1) * 128, :])
        betaW_sb = consts.tile([128, 2, 3 * F], bf16)
        for k in range(2):
            nc.sync.dma_start(out=betaW_sb[:, k, :], in_=betaWT[k * 128:(k + 1) * 128, :])
        testW_sb = consts.tile([128, 3, F], bf16)
        for k in range(3):
            nc.sync.dma_start(out=testW_sb[:, k, :], in_=testWT[k * 128:(k + 1) * 128, :])

        nlagb_sb = consts.tile([128, 2], f32)
        nc.sync.dma_start(out=nlagb_sb[:], in_=nlagb[:, :])
        rbetab_sb = consts.tile([128, 1], f32)
        nc.sync.dma_start(out=rbetab_sb[:], in_=rbeta_b[:, :])
        betab_sb = consts.tile([128, 3], f32)
        nc.sync.dma_start(out=betab_sb[:], in_=beta_b[:, :])
        testb_sb = consts.tile([128, 1], f32)
        nc.sync.dma_start(out=testb_sb[:], in_=test_b[:, :])
        linbr_sb = consts.tile([1, F], bf16)
        nc.sync.dma_start(out=linbr_sb[:], in_=linb_row[:, :])
        zbr_sb = consts.tile([1, F], bf16)
        nc.sync.dma_start(out=zbr_sb[:], in_=zb_row[:, :])
        ones1 = consts.tile([1, BL], bf16)
        nc.vector.memset(ones1[:], 1.0)
        fill_w = consts.tile([128, 128], bf16)
        nc.vector.memset(fill_w[:], 0.0)

        # ---- SBUF-resident phase-P products ----
        big = ctx.enter_context(tc.tile_pool(name="big", bufs=1))
        rr_sb = [big.tile([128, NCH * 2 * BL], bf16, tag=f"rr{j}", name=f"rr_sb{j}") for j in range(NJ)]
        beta_sb = [big.tile([128, NCH * BL], bf16, tag=f"bt{j}", name=f"beta_sb{j}") for j in range(NJ)]

        # =========================== PHASE P ===========================
        with ExitStack() as pctx:
            pstr = pctx.enter_context(tc.tile_pool(name="pstr", bufs=2))
            pwork = pctx.enter_context(tc.tile_pool(name="pwork", bufs=2))
            pps = pctx.enter_context(tc.tile_pool(name="pps", bufs=2, space="PSUM"))

            for j in range(NJ):
                t0 = j * NCH
                lt_ch = pstr.tile([128, NCH, BL], bf16, tag="lt_ch")
                mp_ch = pstr.tile([128, NCH, BL], bf16, tag="mp_ch")
                nc.sync.dma_start(out=lt_ch[:], in_=lt[:, t0:t0 + NCH, :])
                nc.sync.dma_start(out=mp_ch[:], in_=mt[:, t0:t0 + NCH, :])

                NH = W // 2  # half-chunk free width (psum tile size)
                SPH = NCH // 2  # steps per half
                NQ = max(1, NH // 512)  # matmul N sub-tiles per psum tile
                QW = NH // NQ  # sub-tile width (<= 512)
                SQ = QW // BL  # steps per sub-tile

                # rr = min(exp(-(lagW@lt + lag_b)), 1), M-tiles k=0,1
                for k in range(2):
                    for h in range(2):
                        ps = pps.tile([128, NH], f32, tag="pp")
                        for q in range(NQ):
                            nc.tensor.matmul(
                                out=ps[:, q * QW:(q + 1) * QW],
                                lhsT=lagW_sb[:, k * 128:(k + 1) * 128],
                                rhs=lt_ch[:, h * SPH + q * SQ:h * SPH + (q + 1) * SQ, :],
                            )
                        e_ch = pwork.tile([128, NH], bf16, tag="e_ch")
                        nc.scalar.activation(e_ch[:], ps[:], AF.Exp,
                                             bias=nlagb_sb[:, k:k + 1], scale=-1.0)
                        dst = rr_sb[j][:]
                        dst = dst.rearrange("p (t c b) -> p t c b", c=2, b=BL)
                        nc.vector.tensor_scalar_min(
                            dst[:, h * SPH:(h + 1) * SPH, k, :],
                            e_ch[:].rearrange("p (t b) -> p t b", b=BL), 1.0)

                # rb = rbetaW @ rr + rbeta_b  (plain [t*64+b] layout)
                rb_ch = pwork.tile([128, NCH, BL], bf16, tag="rb_ch")
                rrv = rr_sb[j][:].rearrange("p (t c b) -> p t c b", c=2, b=BL)
                for h in range(2):
                    ps = pps.tile([128, NH], f32, tag="pp")
                    for q in range(NQ):
                        for k in range(2):
                            nc.tensor.matmul(
                                out=ps[:, q * QW:(q + 1) * QW],
                                lhsT=rbetaW_sb[:, k, :],
                                rhs=rrv[:, h * SPH + q * SQ:h * SPH + (q + 1) * SQ, k, :],
                                start=(k == 0), stop=(k == 1),
                            )
                    nc.vector.tensor_scalar_add(
                        rb_ch[:, h * SPH:(h + 1) * SPH, :],
                        ps[:].rearrange("p (t b) -> p t b", b=BL),
                        rbetab_sb[:, 0:1])

                # s3 = sigmoid(betaW @ [m; rb] + beta_b), M-tiles m3=0..2
                s3_ch = pwork.tile([128, 3, NCH, BL], bf16, tag="s3_ch")
                for m3 in range(3):
                    for h in range(2):
                        ps = pps.tile([128, NH], f32, tag="pp")
                        for q in range(NQ):
                            for k, rsrc in ((0, mp_ch), (1, rb_ch)):
                                nc.tensor.matmul(
                                    out=ps[:, q * QW:(q + 1) * QW],
                                    lhsT=betaW_sb[:, k, m3 * 128:(m3 + 1) * 128],
                                    rhs=rsrc[:, h * SPH + q * SQ:h * SPH + (q + 1) * SQ, :],
                                    start=(k == 0), stop=(k == 1),
                                )
                        nc.scalar.activation(
                            s3_ch[:, m3, h * SPH:(h + 1) * SPH, :],
                            ps[:].rearrange("p (t b) -> p t b", b=BL),
                            AF.Sigmoid, bias=betab_sb[:, m3:m3 + 1], scale=1.0)

                # beta = testW @ s3 + test_b
                btv = beta_sb[j][:].rearrange("p (t b) -> p t b", b=BL)
                for h in range(2):
                    ps = pps.tile([128, NH], f32, tag="pp")
                    for q in range(NQ):
                        for k in range(3):
                            nc.tensor.matmul(
                                out=ps[:, q * QW:(q + 1) * QW],
                                lhsT=testW_sb[:, k, :],
                                rhs=s3_ch[:, k, h * SPH + q * SQ:h * SPH + (q + 1) * SQ, :],
                                start=(k == 0), stop=(k == 2),
                            )
                    nc.vector.tensor_scalar_add(
                        btv[:, h * SPH:(h + 1) * SPH, :],
                        ps[:].rearrange("p (t b) -> p t b", b=BL),
                        testb_sb[:, 0:1])

        # =========================== PHASE L ===========================
        state = ctx.enter_context(tc.tile_pool(name="state", bufs=2))
        h_bf = state.tile([128, 2 * BL], bf16, tag="h")
        c_f = state.tile([128, 2 * BL], f32, tag="c")
        hr_bf = state.tile([128, 2 * BL], bf16, tag="hr")
        nc.vector.memset(h_bf[:], 0.0)
        nc.vector.memset(c_f[:], 0.0)
        nc.vector.memset(hr_bf[:], 0.0)

        streams = ctx.enter_context(tc.tile_pool(name="streams", bufs=2))
        work = ctx.enter_context(tc.tile_pool(name="work", bufs=3))
        psgif_pool = ctx.enter_context(tc.tile_pool(name="psgif", bufs=2, space="PSUM"))
        psgog_pool = ctx.enter_context(tc.tile_pool(name="psgog", bufs=2, space="PSUM"))
        psl_pool = ctx.enter_context(tc.tile_pool(name="psl", bufs=2, space="PSUM"))
        psz_pool = ctx.enter_context(tc.tile_pool(name="psz", bufs=2, space="PSUM"))

        def make_chunk(j):
            """DMA + mask algebra for chunk j (issued ahead of its use)."""
            t0 = j * NCH
            x_ch = streams.tile([128, NCH, BL], bf16, tag="x_ch")
            m_ch = streams.tile([128, NCH, BL], bf16, tag="m_ch")
            nc.sync.dma_start(out=x_ch[:], in_=xt[:, t0:t0 + NCH, :])
            nc.sync.dma_start(out=m_ch[:], in_=mt[:, t0:t0 + NCH, :])
            im_ch = streams.tile([128, NCH, BL], bf16, tag="im_ch")
            nc.vector.tensor_scalar(im_ch[:], m_ch[:], -1.0, 1.0,
                                    mybir.AluOpType.mult, mybir.AluOpType.add)
            xm_ch = streams.tile([128, NCH, BL], bf16, tag="xm_ch")
            nc.vector.tensor_mul(xm_ch[:], m_ch[:], x_ch[:])
            bview = beta_sb[j][:].rearrange("p (t b) -> p t b", b=BL)
            ib_ch = streams.tile([128, NCH, BL], bf16, tag="ib_ch")
            nc.vector.tensor_mul(ib_ch[:], im_ch[:], bview)
            a_ch = streams.tile([128, NCH, BL], bf16, tag="a_ch")
            nc.vector.tensor_sub(a_ch[:], im_ch[:], ib_ch[:])
            return (m_ch, im_ch, xm_ch, a_ch, ib_ch)

        cur_ch = make_chunk(0)
        nxt_ch = None

        for t in range(T):
            j, tl = t // NCH, t % NCH
            if tl == 0 and t > 0:
                cur_ch = nxt_ch
            if tl == NCH - 6 and j + 1 < NJ:
                nxt_ch = make_chunk(j + 1)
            m_ch, im_ch, xm_ch, a_ch, ib_ch = cur_ch
            ms = m_ch[:, tl, :]
            im_s = im_ch[:, tl, :]
            xm_s = xm_ch[:, tl, :]
            a_s = a_ch[:, tl, :]
            ib_s = ib_ch[:, tl, :]
            rr_t = rr_sb[j][:, tl * 2 * BL:(tl + 1) * 2 * BL]
            beta_t = beta_sb[j][:, tl * BL:(tl + 1) * BL]

            # ---- out = linW @ h + lin_b  (bias via K=1 ones-row matmul) ----
            ps_lin = psl_pool.tile([128, BL], f32, tag="ps_lin")
            nc.tensor.matmul(out=ps_lin[:], lhsT=linbr_sb[:], rhs=ones1[:],
                             start=True, stop=False, skip_group_check=True)
            for k in range(2):
                nc.tensor.matmul(
                    out=ps_lin[:], lhsT=linW_sb[:, k, :],
                    rhs=h_bf[:, k * BL:(k + 1) * BL],
                    start=False, stop=(k == 1), skip_group_check=True,
                )

            # ---- x_c = m*x + (1-m)*out ; z = zod @ x_c + z_b ----
            # z is accumulated as zod@xm (early) + zod@((1-m)*out) (late);
            # w1/w2 are issued before the zod@xc0 matmul so the DVE computes
            # them while the PE finishes z (w2 ready before w3).
            ps_z = psz_pool.tile([128, BL], f32, tag="ps_z")
            nc.tensor.matmul(out=ps_z[:], lhsT=zbr_sb[:], rhs=ones1[:],
                             start=True, stop=False, skip_group_check=True)
            nc.tensor.matmul(out=ps_z[:], lhsT=zod_sb[:], rhs=xm_s,
                             start=False, stop=False, skip_group_check=True)
            xc0 = work.tile([128, BL], bf16, tag="xc0")
            nc.vector.tensor_mul(xc0[:], im_s, ps_lin[:])
            w1 = work.tile([128, BL], f32, tag="w1")
            nc.vector.tensor_mul(w1[:], a_s, ps_lin[:])
            w2 = work.tile([128, BL], bf16, tag="w2")
            nc.vector.tensor_add(w2[:], w1[:], xm_s)
            nc.tensor.matmul(out=ps_z[:], lhsT=zod_sb[:], rhs=xc0[:],
                             start=False, stop=True, skip_group_check=True)
            w3 = work.tile([128, BL], bf16, tag="w3")
            nc.vector.tensor_mul(w3[:], ib_s, ps_z[:])

            # ---- off-chain: out/z to SBUF, c_val = out + beta*(z-out) ----
            out_f = work.tile([128, BL], f32, tag="out_f")
            nc.vector.tensor_copy(out_f[:], ps_lin[:])
            z_f = work.tile([128, BL], f32, tag="z_f")
            nc.scalar.activation(z_f[:], ps_z[:], AF.Copy)
            d_f = work.tile([128, BL], f32, tag="d_f")
            nc.vector.tensor_sub(d_f[:], ps_z[:], out_f[:])
            e_f = work.tile([128, BL], f32, tag="e_f")
            nc.gpsimd.tensor_mul(e_f[:], beta_t, d_f[:])
            cval_f = work.tile([128, BL], f32, tag="cval_f")
            nc.gpsimd.tensor_add(cval_f[:], e_f[:], out_f[:])

            # ---- gates = bias + Wih@[cc; m] + Whh@h_rr  (order i,f,o,g) ----
            # i,f columns (0-3) in ps_gIF, o,2g columns (4-7) in ps_gOG —
            # separate PSUM banks so sigmoid(i,f) fires as soon as its own
            # columns finish.  g rows were pre-scaled x2 host-side, so one
            # sigmoid table serves all gates: tanh(g) = 2*sig(2g) - 1.
            ps_gIF = psgif_pool.tile([128, 4 * BL], f32, tag="ps_gIF")
            ps_gOG = psgog_pool.tile([128, 4 * BL], f32, tag="ps_gOG")
            nc.tensor.matmul(out=ps_gIF[:], lhsT=bias8_sb[:],
                             rhs=sel8_sb[:, 0:4 * BL],
                             start=True, stop=False, skip_group_check=True)
            nc.tensor.matmul(out=ps_gOG[:], lhsT=bias8_sb[:],
                             rhs=sel8_sb[:, 4 * BL:8 * BL],
                             start=True, stop=False, skip_group_check=True)

            def gmm(lhs, mc, rhs, stop):
                out = (ps_gIF[:, mc * BL:(mc + 1) * BL] if mc < 4 else
                       ps_gOG[:, (mc - 4) * BL:(mc - 3) * BL])
                nc.tensor.matmul(out=out, lhsT=lhs[:, mc * 128:(mc + 1) * 128],
                                 rhs=rhs, start=False, stop=stop,
                                 skip_group_check=True)

            grhs = (ms, hr_bf[:, 0:BL], hr_bf[:, BL:2 * BL])
            glhs = (Wih_sb[:, 1, :], Whh_sb[:, 0, :], Whh_sb[:, 1, :])
            for k in range(3):
                for mc in range(8):
                    gmm(glhs[k], mc, grhs[k], False)
            # w2/w3 per column (shared stationary), i/f columns first
            for mc in range(8):
                gmm(Wih_sb[:, 0, :], mc, w2[:], False)
                gmm(Wih_sb[:, 0, :], mc, w3[:], True)

            # ---- PE warm-keepers (optional) ----
            for _ in range(NFILL):
                nc.tensor.ldweights(weights=fill_w[:])

            # ---- gate activations: one sigmoid table for everything ----
            sif = work.tile([128, 4 * BL], f32, tag="sif")
            nc.scalar.activation(sif[:], ps_gIF[:], AF.Sigmoid)
            sog = work.tile([128, 4 * BL], f32, tag="sog")
            nc.scalar.activation(sog[:], ps_gOG[:], AF.Sigmoid)

            # ---- C = c'/2 = sig_f*C + (s-1/2)*sig_i;  h' = sig_o*tanh(2C)
            t1 = work.tile([128, 2 * BL], f32, tag="t1")
            nc.vector.tensor_mul(t1[:], sif[:, 2 * BL:4 * BL], c_f[:])
            t2 = work.tile([128, 2 * BL], f32, tag="t2")
            nc.vector.scalar_tensor_tensor(
                out=t2[:], in0=sog[:, 2 * BL:4 * BL], scalar=0.5,
                in1=sif[:, 0:2 * BL], op0=mybir.AluOpType.subtract,
                op1=mybir.AluOpType.mult)
            c_new = state.tile([128, 2 * BL], f32, tag="c")
            nc.vector.tensor_add(c_new[:], t1[:], t2[:])
            tc2 = work.tile([128, 2 * BL], f32, tag="tc2")
            nc.scalar.activation(tc2[:], c_new[:], AF.Tanh, scale=2.0)
            h_new = state.tile([128, 2 * BL], bf16, tag="h")
            nc.vector.tensor_mul(h_new[:], sog[:, 0:2 * BL], tc2[:])
            if t + 1 < T:
                jn, tn = (t + 1) // NCH, (t + 1) % NCH
                rr_n = rr_sb[jn][:, tn * 2 * BL:(tn + 1) * 2 * BL]
                or2 = work.tile([128, 2 * BL], f32, tag="or2")
                nc.gpsimd.tensor_mul(or2[:], sog[:, 0:2 * BL], rr_n)
                hr_new = state.tile([128, 2 * BL], bf16, tag="hr")
                nc.gpsimd.tensor_mul(hr_new[:], or2[:], tc2[:])
                hr_bf = hr_new

            # ---- store outputs ----
            nc.sync.dma_start(out=o_out[t], in_=out_f[:])
            nc.sync.dma_start(out=z_out[t], in_=z_f[:])
            nc.sync.dma_start(out=c_out[t], in_=cval_f[:])

            h_bf = h_new
            c_f = c_new

    nc.compile()
    return nc


def _prep_weights(inputs, d):
    """Host-side weight layout prep for direction d (0=fw, 1=bw). bf16."""
    p = "fw" if d == 0 else "bw"
    Wih = np.asarray(inputs[f"{p}_Wih"], np.float32)
    Whh = np.asarray(inputs[f"{p}_Whh"], np.float32)
    bih = np.asarray(inputs[f"{p}_bih"], np.float32)
    bhh = np.asarray(inputs[f"{p}_bhh"], np.float32)
    lin_W = np.asarray(inputs[f"{p}lin_W"], np.float32)
    lin_b = np.asarray(inputs[f"{p}lin_b"], np.float32)
    z_W = np.asarray(inputs[f"{p}z_W"], np.float32)
    z_b = np.asarray(inputs[f"{p}z_b"], np.float32)
    beta_W = np.asarray(inputs[f"{p}beta_W"], np.float32)
    beta_b = np.asarray(inputs[f"{p}beta_b"], np.float32)
    lag_W = np.asarray(inputs["lag_W" if d == 0 else "lagb_W"], np.float32)
    lag_b = np.asarray(inputs["lag_b" if d == 0 else "lagb_b"], np.float32)
    rbeta_W = np.asarray(inputs["rbeta_W" if d == 0 else "rbetab_W"], np.float32)
    rbeta_b = np.asarray(inputs["rbeta_b" if d == 0 else "rbetab_b"], np.float32)
    test_W = np.asarray(inputs["test_W"], np.float32)
    test_b = np.asarray(inputs["test_b"], np.float32)

    perm = np.concatenate([np.arange(0, 512), np.arange(768, 1024),
                           np.arange(512, 768)])
    sel8 = np.zeros((8, 8 * BL), np.float32)
    for j in range(8):
        sel8[j, j * BL:(j + 1) * BL] = 1.0

    def c(a):
        return np.ascontiguousarray(a)

    # scale the g-gate rows x2 so tanh(g) = 2*sigmoid(2g)-1 can reuse the
    # sigmoid activation (gate order after perm: i, f, o, g)
    gsc = np.ones((1024, 1), np.float32)
    gsc[768:] = 2.0
    Wih_p = Wih[perm] * gsc
    Whh_p = Whh[perm] * gsc
    bias_p = (bih + bhh)[perm] * gsc[:, 0]

    w = {
        "linWT": c(lin_W.T).astype(_BF),
        "zodT": c((z_W * (1.0 - np.eye(F, dtype=np.float32))).T).astype(_BF),
        "WihT": c(Wih_p.T).astype(_BF),
        "WhhT": c(Whh_p.T).astype(_BF),
        "bias8": c(bias_p.reshape(8, F)).astype(_BF),
        "sel8": sel8.astype(_BF),
        "lagWT": c(lag_W.T).astype(_BF),
        "nlagb": c((-lag_b).reshape(2, F).T).astype(np.float32),
        "rbetaWT": c(rbeta_W.T).astype(_BF),
        "rbeta_b": c(rbeta_b.reshape(F, 1)).astype(np.float32),
        "betaWT": c(beta_W.T).astype(_BF),
        "beta_b": c(beta_b.reshape(3, F).T).astype(np.float32),
        "testWT": c(test_W.T).astype(_BF),
        "test_b": c(test_b.reshape(F, 1)).astype(np.float32),
        "linb_row": c(lin_b.reshape(1, F)).astype(_BF),
        "zb_row": c(z_b.reshape(1, F)).astype(_BF),
    }
    return w


def _make_in_maps(inputs, T):
    x = np.asarray(inputs["x"], np.float32)
    m = np.asarray(inputs["masking"], np.float32)
    tl = np.asarray(inputs["time_lag"], np.float32)
    Bfull = x.shape[0]
    nsh = Bfull // BL

    xt = np.ascontiguousarray(x.transpose(2, 1, 0)).astype(_BF)
    mt = np.ascontiguousarray(m.transpose(2, 1, 0)).astype(_BF)
    ltt = np.ascontiguousarray(tl.transpose(2, 1, 0)).astype(_BF)
    xtr = np.ascontiguousarray(xt[:, ::-1, :])
    mtr = np.ascontiguousarray(mt[:, ::-1, :])
    lttr = np.ascontiguousarray(ltt[:, ::-1, :])

    wts = [_prep_weights(inputs, 0), _prep_weights(inputs, 1)]

    in_maps = []
    for core in range(NCORES):
        d, s = core // nsh, core % nsh
        sl = slice(s * BL, (s + 1) * BL)
        if d == 0:
            im = {"xt": np.ascontiguousarray(xt[:, :, sl]),
                  "mt": np.ascontiguousarray(mt[:, :, sl]),
                  "lt": np.ascontiguousarray(ltt[:, :, sl])}
        else:
            im = {"xt": np.ascontiguousarray(xtr[:, :, sl]),
                  "mt": np.ascontiguousarray(mtr[:, :, sl]),
                  "lt": np.ascontiguousarray(lttr[:, :, sl])}
        im.update(wts[d])
        in_maps.append(im)
    return in_maps, nsh


def _gather(res, T, Bfull, nsh):
    outs = []
    for d in range(2):
        o = np.empty((Bfull, T, F), np.float32)
        z = np.empty((Bfull, T, F), np.float32)
        cv = np.empty((Bfull, T, F), np.float32)
        for s in range(nsh):
            r = res[d * nsh + s]
            sl = slice(s * BL, (s + 1) * BL)
            ro, rz, rc = r["o_out"], r["z_out"], r["c_out"]
            if d == 1:
                ro, rz, rc = ro[::-1], rz[::-1], rc[::-1]
            o[sl] = ro.transpose(2, 0, 1)
            z[sl] = rz.transpose(2, 0, 1)
            cv[sl] = rc.transpose(2, 0, 1)
        outs += [o, z, cv]
    return tuple(outs)


def _run(inputs, T, trace=False):
    from concourse.bass_utils import run_bass_kernel_spmd

    if T not in _BUILD_CACHE:
        _BUILD_CACHE[T] = _build(T)
    nc = _BUILD_CACHE[T]
    in_maps, nsh = _make_in_maps(inputs, T)
    br = run_bass_kernel_spmd(nc, in_maps, core_ids=list(range(NCORES)),
                              trace=trace)
    Bfull = np.asarray(inputs["x"]).shape[0]
    return _gather(br.results, T, Bfull, nsh), br


def kernel(**inputs):
    T = np.asarray(inputs["x"]).shape[1]
    outs, _ = _run(inputs, T, trace=False)
    return outs

